# revision 17
# baseline (speedup 1.0000x reference)
"""Trainium2 Bass kernel for causal self-attention with RoPE.

Shapes: x (2, 2048, 2048), 16 heads x 128 head_dim.
Sharding: 8 cores = 2 batch x 4 head-groups (4 heads per core).
Each core computes q/k/v projections for its heads, RoPE, causal-masked
softmax attention, and a partial output projection (its head columns of
wo); the host sums the 4 partials per batch element.

Layout strategy (per core):
  - all matmul operands are bf16 (fp32 PSUM accumulation); the loose
    2e-2 tolerance leaves ample margin and bf16 halves DMA traffic and
    eases PE power throttling relative to fp32r.
  - q,k built in transposed layout (head_dim on partitions, t free) so
    RoPE and the score matmuls need no on-device transposes.  The host
    permutes wq/wk columns so RoPE's even/odd pairs become the two
    partition halves, and pre-scales wq by 1/sqrt(head_dim).
  - q/k/v projections run in ONE pass over x^T (v's matmuls use the
    x^T chunks as the stationary operand), so x^T streams from HBM
    once instead of twice.
  - scores computed as s^T (keys x q) per 256-query group; softmax skips
    the max-subtraction (scores are O(1) by construction); row sums via
    an ones-vector matmul; normalization folded into the PSUM eviction.
  - attention runs a 2-deep software pipeline over key-chunk PAIRS so
    the ScalarE exp latency is fully hidden behind the p@v matmuls of
    older pairs; each (query-group, head)'s o and l accumulate in one
    shared PSUM bank to stay within the 8-bank budget.
  - fully-masked key blocks are skipped (host inspects the mask);
    deduplicated exp-mask tiles are multiplied into p only where a
    block is partially masked (exp(s+m) == exp(s)*exp(m)).
  - DMA is spread over four hardware queues (per-queue throughput is
    ~100 GB/s): wq on scalar, wk on vector, wv on gpsimd, x^T/tables on
    sync; the output round-robins over sync/vector/gpsimd.
"""

import sys
from collections import deque
from contextlib import ExitStack

if "/opt/trn_rl_repo" not in sys.path:
    sys.path.insert(0, "/opt/trn_rl_repo")

import numpy as np
from ml_dtypes import bfloat16

import concourse.bacc as bacc
import concourse.mybir as mybir
import concourse.tile as tile
from concourse.bass_utils import run_bass_kernel_spmd

B, T, D, NH, HD = 2, 2048, 2048, 16, 128
HPC = 4              # heads per core
PAIR = 256           # queries per group
NPAIR = T // PAIR    # 8
NCHUNK = T // HD     # 16 key chunks of 128
NSLICE = T // PAIR   # 8 t-slices for projections
BF16 = mybir.dt.bfloat16
F32 = mybir.dt.float32
MASK_PRELOAD_MAX = 24
DEPTH = 2            # attention pair-pipeline lookahead


def _mask_structure(mask):
    """Classify each (query-group, key-chunk) block of the additive mask.

    Returns (statuses, maskt): statuses[j] is a list of
    (chunk, mask_tile_index_or_minus1), in reversed chunk order (the
    attention loop consumes them in that order, so dedup indices of
    adjacent partial blocks come out consecutive), for blocks that must
    be computed; maskt is the packed (128, nmask, 256) array of
    deduplicated transposed exp-mask tiles for partially-masked blocks.
    """
    statuses = []
    tiles = {}
    tile_list = []
    for j in range(NPAIR):
        q = slice(j * PAIR, (j + 1) * PAIR)
        lst = []
        for c in reversed(range(NCHUNK)):
            k = slice(c * HD, (c + 1) * HD)
            sub = mask[q, k]
            if np.all(sub <= -1e8):
                continue
            if np.all(sub == 0.0):
                lst.append((c, -1))
            else:
                key = sub.tobytes()
                mi = tiles.get(key)
                if mi is None:
                    mi = len(tile_list)
                    tiles[key] = mi
                    tile_list.append(np.ascontiguousarray(sub.T))
                lst.append((c, mi))
        assert lst, f"query group {j} has every key block masked"
        statuses.append(lst)
    nmask = max(1, len(tile_list))
    maskt = np.zeros((HD, nmask, PAIR), np.float32)
    for i, t in enumerate(tile_list):
        assert np.all(t <= 64.0), "additive mask too large for exp-mask trick"
        maskt[:, i, :] = np.exp(t)
    return statuses, maskt


def _build_program(statuses, nmask):
    nc = bacc.Bacc(None, target_bir_lowering=False)

    xt_d = nc.dram_tensor("xt", [D, T], BF16, kind="ExternalInput")
    wq_d = nc.dram_tensor("wqt", [D, HPC * HD], BF16, kind="ExternalInput")
    wk_d = nc.dram_tensor("wkt", [D, HPC * HD], BF16, kind="ExternalInput")
    wv_d = nc.dram_tensor("wvt", [D, HPC * HD], BF16, kind="ExternalInput")
    wo_d = nc.dram_tensor("wot", [HPC * HD, D], BF16, kind="ExternalInput")
    cs_d = nc.dram_tensor("cs", [HD, 2, T], F32, kind="ExternalInput")
    mk_d = nc.dram_tensor("maskt", [HD, nmask, PAIR], BF16, kind="ExternalInput")
    ones_d = nc.dram_tensor("ones_col", [HD, 1], BF16, kind="ExternalInput")
    out_d = nc.dram_tensor("out", [T, D], BF16, kind="ExternalOutput")

    xt_ap = xt_d.ap().rearrange("(k p) t -> p k t", p=HD)
    wq_ap = wq_d.ap().rearrange("(k p) e -> p k e", p=HD)
    wk_ap = wk_d.ap().rearrange("(k p) e -> p k e", p=HD)
    wv_ap = wv_d.ap().rearrange("(k p) e -> p k e", p=HD)
    wo_ap = wo_d.ap().rearrange("(h p) e -> p h e", p=HD)
    EXP = mybir.ActivationFunctionType.Exp
    preload_mask = nmask <= MASK_PRELOAD_MAX

    with tile.TileContext(nc) as tc, ExitStack() as top:
        constp = top.enter_context(tc.tile_pool(name="const", bufs=1))
        ones_sb = constp.tile([HD, 1], BF16)
        scr_sb = constp.tile([1, 2], F32)
        nc.scalar.dma_start(ones_sb[:], ones_d[:])
        # dummy exp so the ScalarE activation table for Exp loads during
        # the projection phase instead of stalling the first real exp
        nc.scalar.activation(scr_sb[0:1, 0:1], ones_sb[0:1, 0:1], EXP)

        qkp = top.enter_context(tc.tile_pool(name="qkp", bufs=1))
        # q heads at [:, h, :], k heads at [:, 4+h, :]
        qk_sb = qkp.tile([HD, 2 * HPC, T], BF16)
        vap = top.enter_context(tc.tile_pool(name="vap", bufs=1))
        v_all = vap.tile([HD, NCHUNK, HPC * HD], BF16)

        # ---- fused q/k/v projection pass (+ fused RoPE) ----
        with ExitStack() as ph:
            wp = ph.enter_context(tc.tile_pool(name="wp", side="right", bufs=1))
            wvp = ph.enter_context(tc.tile_pool(name="wvp", side="right", bufs=1))
            xtp = ph.enter_context(tc.tile_pool(name="xtp", side="right", bufs=3))
            csp = ph.enter_context(tc.tile_pool(name="csp", side="right", bufs=2))
            ropep = ph.enter_context(tc.tile_pool(name="ropep", side="right", bufs=2))
            pps = ph.enter_context(tc.tile_pool(name="pps", bufs=5, space="PSUM"))
            vps = ph.enter_context(tc.tile_pool(name="vps", bufs=2, space="PSUM"))
            wqk_sb = wp.tile([HD, 2, NCHUNK, HPC * HD], BF16)
            wv_sb = wvp.tile([HD, NCHUNK, HPC * HD], BF16)
            # weights split per k-chunk (first matmuls start as soon as
            # chunk 0 lands), even/odd chunks spread over two DMA queues
            # (each hardware queue tops out near 100 GB/s)
            wqs = [nc.scalar, nc.gpsimd]
            for k in range(NCHUNK):
                wqs[k % 2].dma_start(wqk_sb[:, 0, k, :], wq_ap[:, k, :])
            for k in range(NCHUNK):
                wqs[k % 2].dma_start(wqk_sb[:, 1, k, :], wk_ap[:, k, :])
            for k in range(NCHUNK):
                wqs[k % 2].dma_start(wv_sb[:, k, :], wv_ap[:, k, :])

            def v_groups(xt, ns):
                # v projection for a slice: x^T chunks stationary, so v
                # lands directly in (t x e) layout
                for tc2 in range(2):
                    psv = vps.tile([HD, HPC * HD], F32, tag="vps")
                    for k in range(NCHUNK):
                        nc.tensor.matmul(
                            psv[:],
                            xt[:, k, tc2 * HD:(tc2 + 1) * HD],
                            wv_sb[:, k, :],
                            start=(k == 0),
                            stop=(k == NCHUNK - 1),
                        )
                    nc.scalar.copy(v_all[:, ns * 2 + tc2, :], psv[:])

            xt0 = None
            for ns in range(NSLICE):
                tsl = slice(ns * PAIR, (ns + 1) * PAIR)
                xt = xtp.tile([HD, NCHUNK, PAIR], BF16, tag="xt")
                if ns == 0:
                    xt0 = xt
                    # quarter the first slice's DMA so matmul 0 starts early
                    for qq in range(4):
                        nc.sync.dma_start(
                            xt[:, qq * 4:(qq + 1) * 4, :],
                            xt_ap[:, qq * 4:(qq + 1) * 4, tsl],
                        )
                else:
                    nc.sync.dma_start(xt[:], xt_ap[:, :, tsl])
                cs_sl = csp.tile([HD, 2, PAIR], F32, tag="cs")
                nc.sync.dma_start(cs_sl[:], cs_d[:, :, tsl])
                for wsel in range(2):
                    for h in range(HPC):
                        ps = pps.tile([HD, PAIR], F32, tag="ps")
                        hs = slice(h * HD, (h + 1) * HD)
                        for k in range(NCHUNK):
                            nc.tensor.matmul(
                                ps[:],
                                wqk_sb[:, wsel, k, hs],
                                xt[:, k, :],
                                start=(k == 0),
                                stop=(k == NCHUNK - 1),
                            )
                        # RoPE: dst = raw*C + swap(raw)*S.  The swap is
                        # materialized by two ScalarE half-copies; DVE does
                        # two multiplies and one add per tile.
                        dst = qk_sb[:, wsel * HPC + h, tsl]
                        sw = ropep.tile([HD, PAIR], F32, tag="sw")
                        nc.scalar.copy(sw[0:64, :], ps[64:128, :])
                        nc.scalar.copy(sw[64:128, :], ps[0:64, :])
                        tb = ropep.tile([HD, PAIR], BF16, tag="tb")
                        nc.vector.tensor_mul(dst, ps[:], cs_sl[:, 0, :])
                        nc.vector.tensor_mul(tb[:], sw[:], cs_sl[:, 1, :])
                        nc.vector.tensor_add(dst, dst, tb[:])
                # slice 0's v groups are deferred until after slice 1 so
                # the wv DMA (behind wq/wk on its queues) is off the
                # startup critical path; attention consumes v chunks in
                # ascending order, so chunks 0/1 are not needed early.
                if ns == 1:
                    v_groups(xt, ns)
                    v_groups(xt0, 0)
                elif ns > 1:
                    v_groups(xt, ns)

        # ---- attention ----
        ctxp = top.enter_context(tc.tile_pool(name="ctxp", bufs=1))
        ctx_sb = ctxp.tile([HD, HPC, T], BF16)
        wop = top.enter_context(tc.tile_pool(name="wop", bufs=1))
        wo_sb = wop.tile([HD, HPC, D], BF16)
        with ExitStack() as ph:
            ptp = ph.enter_context(tc.tile_pool(name="ptp", side="right", bufs=3))
            mkp = ph.enter_context(tc.tile_pool(name="mkp", side="right", bufs=4))
            lrp = ph.enter_context(tc.tile_pool(name="lrp", side="right", bufs=2))
            rbp = ph.enter_context(tc.tile_pool(name="rbp", side="right", bufs=2))
            sps = ph.enter_context(tc.tile_pool(name="sps", bufs=4, space="PSUM"))
            ops = ph.enter_context(tc.tile_pool(name="ops", bufs=2, space="PSUM"))
            lps = ph.enter_context(tc.tile_pool(name="lps", bufs=2, space="PSUM"))

            mk_sb = None
            if preload_mask:
                mkpre = ph.enter_context(
                    tc.tile_pool(name="mkpre", side="right", bufs=1)
                )
                mk_sb = mkpre.tile([HD, nmask, PAIR], BF16)
                nc.scalar.dma_start(mk_sb[:], mk_d[:])
            for h in range(HPC):  # prefetch wo
                nc.scalar.dma_start(wo_sb[:, h, :], wo_ap[:, h, :])

            def mask_tile(mi):
                if preload_mask:
                    return mk_sb[:, mi, :]
                mt = mkp.tile([HD, PAIR], BF16, tag="mk")
                nc.scalar.dma_start(mt[:], mk_d[:, mi, :])
                return mt[:]

            def finalize(st):
                # off the tensor engine: DVE fast-recip -> GpSimd partition
                # broadcast -> DVE multiply into ctx
                lr = lrp.tile([1, PAIR], F32, tag="lr")
                nc.vector.reciprocal_approx_fast(lr[:], st["l"])
                rb_sb = rbp.tile([HD, PAIR], F32, tag="rb")
                nc.gpsimd.partition_broadcast(rb_sb[:], lr[:])
                nc.vector.tensor_mul(
                    ctx_sb[:, st["h"], st["qsl"]], st["o"], rb_sb[:]
                )

            def emit_ol(dq):
                # deferred p@v and row-sum matmuls for an exp'd batch.
                # All o matmuls stream first, then all l matmuls, so the
                # PE's moving-operand source stays on pt for the whole
                # run (source switches cost ~95ns each).
                pi, batch, st = dq
                h = st["h"]
                for t, (c, mi) in enumerate(batch):
                    nc.tensor.matmul(
                        st["o"],
                        v_all[:, c, h * HD:(h + 1) * HD],
                        st["pt"][:, pi + t, :],
                        start=(st["oi"] == 0),
                        stop=(st["oi"] == st["n"] - 1),
                        skip_group_check=True,
                    )
                    st["oi"] += 1
                for t, (c, mi) in enumerate(batch):
                    nc.tensor.matmul(
                        st["l"],
                        ones_sb[:],
                        st["pt"][:, pi + t, :],
                        start=(st["li"] == 0),
                        stop=(st["li"] == st["n"] - 1),
                        skip_group_check=True,
                    )
                    st["li"] += 1
                return st["li"] == st["n"]

            pend = deque()   # batches whose ol matmuls are deferred
            fin_q = []       # sts whose ol is fully emitted, finalize pending

            def pop_one():
                dq = pend.popleft()
                if emit_ol(dq):
                    fin_q.append(dq[2])

            def pend_chunks():
                return sum(len(dq[1]) for dq in pend)

            BATCH = 8  # chunks per pipeline batch (4 PSUM score banks)
            for j in range(NPAIR):
                qsl = slice(j * PAIR, (j + 1) * PAIR)
                chunks = statuses[j]  # already in reversed chunk order
                n = len(chunks)
                batches = [chunks[ii:ii + BATCH] for ii in range(0, n, BATCH)]
                for h in range(HPC):
                    o_ps = ops.tile([HD, PAIR], F32, tag="o")
                    l_ps = lps.tile([1, PAIR], F32, tag="l")
                    pt = ptp.tile([HD, NCHUNK, PAIR], BF16, tag="pt")
                    st = {"o": o_ps[:], "l": l_ps[:],
                          "pt": pt, "h": h, "qsl": qsl, "n": n,
                          "oi": 0, "li": 0}
                    pi = 0
                    for batch in batches:
                        w = len(batch)
                        # scores: one long qk_sb-sourced run, split into
                        # per-pair PSUM tiles so exp can chase the matmuls
                        stiles = []
                        for ii in range(0, w, 2):
                            pr = batch[ii:ii + 2]
                            s_ps = sps.tile([HD, 2, PAIR], F32, tag="s")
                            stiles.append((ii, pr, s_ps))
                            for t, (c, mi) in enumerate(pr):
                                nc.tensor.matmul(
                                    s_ps[:, t, :],
                                    qk_sb[:, HPC + h, c * HD:(c + 1) * HD],
                                    qk_sb[:, h, qsl],
                                    start=True,
                                    stop=True,
                                )
                        for ii, pr, s_ps in stiles:
                            wp2 = len(pr)
                            nc.scalar.activation(
                                pt[:, pi + ii:pi + ii + wp2, :],
                                s_ps[:, 0:wp2, :], EXP,
                            )
                            # multiplicative exp-mask applied to pt
                            # (exp(s+m) == exp(s)*exp(m)), off the exp chain
                            t = 0
                            while t < wp2:
                                c, mi = pr[t]
                                if mi < 0:
                                    t += 1
                                    continue
                                r = t + 1
                                while (preload_mask and r < wp2
                                       and pr[r][1] >= 0
                                       and pr[r][1] == pr[r - 1][1] + 1):
                                    r += 1
                                if preload_mask:
                                    sl = slice(pi + ii + t, pi + ii + r)
                                    nc.vector.tensor_mul(
                                        pt[:, sl, :], pt[:, sl, :],
                                        mk_sb[:, mi:mi + (r - t), :],
                                    )
                                else:
                                    sl = slice(pi + ii + t, pi + ii + t + 1)
                                    nc.vector.tensor_mul(
                                        pt[:, sl, :], pt[:, sl, :],
                                        mask_tile(mi),
                                    )
                                    r = t + 1
                                t = r
                        while fin_q:
                            finalize(fin_q.pop(0))
                        # keep ~one full batch of exp'd chunks in flight:
                        # short (small-j) batches accumulate so the PE
                        # always has deferred ol work to hide exp latency
                        while pend_chunks() >= BATCH or len(pend) >= 4:
                            pop_one()
                        pend.append((pi, batch, st))
                        pi += w
            while pend:
                pop_one()
            while fin_q:
                finalize(fin_q.pop(0))
        # ---- output projection (forward order: with ascending-j
        # attention, low t-chunks finished longest ago) ----
        with ExitStack() as ph:
            evp = ph.enter_context(tc.tile_pool(name="evp", side="right", bufs=6))
            wops = ph.enter_context(tc.tile_pool(name="wops", bufs=6, space="PSUM"))
            oqs = [nc.sync, nc.scalar, nc.gpsimd]
            for oi, tck in enumerate(range(NCHUNK)):
                tsl = slice(tck * HD, (tck + 1) * HD)
                for es in range(4):
                    esl = slice(es * 512, (es + 1) * 512)
                    ps = wops.tile([HD, 512], F32, tag="wo")
                    for h in range(HPC):
                        nc.tensor.matmul(
                            ps[:],
                            ctx_sb[:, h, tsl],
                            wo_sb[:, h, esl],
                            start=(h == 0),
                            stop=(h == HPC - 1),
                        )
                    ev = evp.tile([HD, 512], BF16, tag="ev")
                    nc.scalar.copy(ev[:], ps[:])
                    oqs[(oi * 4 + es) % 3].dma_start(out_d[tsl, esl], ev[:])
    nc.compile()
    return nc


_PERM = np.concatenate(
    [np.concatenate([np.arange(0, HD, 2), np.arange(1, HD, 2)]) + h * HD
     for h in range(HPC)]
)


def prepare(x, freqs, mask, wq, wk, wv, wo):
    """Host-side sharding/prep. Returns (nc, in_maps)."""
    x = np.asarray(x, np.float32)
    freqs = np.asarray(freqs, np.float32)
    mask = np.asarray(mask, np.float32)
    wq, wk, wv, wo = (np.asarray(w, np.float32) for w in (wq, wk, wv, wo))

    statuses, maskt = _mask_structure(mask)
    nc = _build_program(statuses, maskt.shape[1])

    scale = np.float32(1.0 / np.sqrt(HD))
    cos = np.ascontiguousarray(freqs[:, :, 0].T)  # (64, T)
    sin = np.ascontiguousarray(freqs[:, :, 1].T)
    cs = np.empty((HD, 2, T), np.float32)
    cs[0:64, 0, :] = cos
    cs[64:128, 0, :] = cos
    cs[0:64, 1, :] = -sin
    cs[64:128, 1, :] = sin

    ones_col = np.ones((HD, 1), bfloat16)
    maskt16 = maskt.astype(bfloat16)
    xt = [np.ascontiguousarray(x[b].T).astype(bfloat16) for b in range(B)]

    in_maps = []
    for core in range(8):
        b, g = core // 4, core % 4
        cols = slice(g * HPC * HD, (g + 1) * HPC * HD)
        in_maps.append({
            "xt": xt[b],
            "wqt": np.ascontiguousarray(
                (wq.T[:, cols] * scale)[:, _PERM]).astype(bfloat16),
            "wkt": np.ascontiguousarray(wk.T[:, cols][:, _PERM]).astype(bfloat16),
            "wvt": np.ascontiguousarray(wv.T[:, cols]).astype(bfloat16),
            "wot": np.ascontiguousarray(wo.T[cols, :]).astype(bfloat16),
            "cs": cs,
            "maskt": maskt16,
            "ones_col": ones_col,
        })
    return nc, in_maps


def run(x, freqs, mask, wq, wk, wv, wo, **spmd_kwargs):
    nc, in_maps = prepare(x, freqs, mask, wq, wk, wv, wo)
    res = run_bass_kernel_spmd(nc, in_maps, list(range(8)), **spmd_kwargs)
    parts = [res.results[c]["out"].astype(np.float32) for c in range(8)]
    out = np.stack([
        parts[b * 4] + parts[b * 4 + 1] + parts[b * 4 + 2] + parts[b * 4 + 3]
        for b in range(B)
    ]).astype(np.float32)
    return out, res


def kernel(x, freqs, mask, wq, wk, wv, wo):
    out, _ = run(x, freqs, mask, wq, wk, wv, wo)
    return out


# revision 20
# speedup vs baseline: 1.0036x; 1.0036x over previous
"""Trainium2 Bass kernel for causal self-attention with RoPE.

Shapes: x (2, 2048, 2048), 16 heads x 128 head_dim.
Sharding: 8 cores = 2 batch x 4 head-groups (4 heads per core).
Each core computes q/k/v projections for its heads, RoPE, causal-masked
softmax attention, and a partial output projection (its head columns of
wo); the host sums the 4 partials per batch element.

Layout strategy (per core):
  - all matmul operands are bf16 (fp32 PSUM accumulation); the loose
    2e-2 tolerance leaves ample margin and bf16 halves DMA traffic and
    eases PE power throttling relative to fp32r.
  - q,k built in transposed layout (head_dim on partitions, t free) so
    RoPE and the score matmuls need no on-device transposes.  The host
    permutes wq/wk columns so RoPE's even/odd pairs become the two
    partition halves, and pre-scales wq by 1/sqrt(head_dim).
  - q/k/v projections run in ONE pass over x^T (v's matmuls use the
    x^T chunks as the stationary operand), so x^T streams from HBM
    once instead of twice.
  - scores computed as s^T (keys x q) per 256-query group; softmax skips
    the max-subtraction (scores are O(1) by construction); row sums via
    an ones-vector matmul; normalization folded into the PSUM eviction.
  - attention runs a 2-deep software pipeline over key-chunk PAIRS so
    the ScalarE exp latency is fully hidden behind the p@v matmuls of
    older pairs; each (query-group, head)'s o and l accumulate in one
    shared PSUM bank to stay within the 8-bank budget.
  - fully-masked key blocks are skipped (host inspects the mask);
    deduplicated exp-mask tiles are multiplied into p only where a
    block is partially masked (exp(s+m) == exp(s)*exp(m)).
  - DMA is spread over four hardware queues (per-queue throughput is
    ~100 GB/s): wq on scalar, wk on vector, wv on gpsimd, x^T/tables on
    sync; the output round-robins over sync/vector/gpsimd.
"""

import sys
from collections import deque
from contextlib import ExitStack

if "/opt/trn_rl_repo" not in sys.path:
    sys.path.insert(0, "/opt/trn_rl_repo")

import numpy as np
from ml_dtypes import bfloat16

import concourse.bacc as bacc
import concourse.mybir as mybir
import concourse.tile as tile
from concourse.bass_utils import run_bass_kernel_spmd

B, T, D, NH, HD = 2, 2048, 2048, 16, 128
HPC = 4              # heads per core
PAIR = 256           # queries per group
NPAIR = T // PAIR    # 8
NCHUNK = T // HD     # 16 key chunks of 128
NSLICE = T // PAIR   # 8 t-slices for projections
BF16 = mybir.dt.bfloat16
F32 = mybir.dt.float32
MASK_PRELOAD_MAX = 24
DEPTH = 2            # attention pair-pipeline lookahead


def _mask_structure(mask):
    """Classify each (query-group, key-chunk) block of the additive mask.

    Returns (statuses, maskt): statuses[j] is a list of
    (chunk, mask_tile_index_or_minus1), in reversed chunk order (the
    attention loop consumes them in that order, so dedup indices of
    adjacent partial blocks come out consecutive), for blocks that must
    be computed; maskt is the packed (128, nmask, 256) array of
    deduplicated transposed exp-mask tiles for partially-masked blocks.
    """
    statuses = []
    tiles = {}
    tile_list = []
    for j in range(NPAIR):
        q = slice(j * PAIR, (j + 1) * PAIR)
        lst = []
        for c in reversed(range(NCHUNK)):
            k = slice(c * HD, (c + 1) * HD)
            sub = mask[q, k]
            if np.all(sub <= -1e8):
                continue
            if np.all(sub == 0.0):
                lst.append((c, -1))
            else:
                key = sub.tobytes()
                mi = tiles.get(key)
                if mi is None:
                    mi = len(tile_list)
                    tiles[key] = mi
                    tile_list.append(np.ascontiguousarray(sub.T))
                lst.append((c, mi))
        assert lst, f"query group {j} has every key block masked"
        statuses.append(lst)
    nmask = max(1, len(tile_list))
    maskt = np.zeros((HD, nmask, PAIR), np.float32)
    for i, t in enumerate(tile_list):
        assert np.all(t <= 64.0), "additive mask too large for exp-mask trick"
        maskt[:, i, :] = np.exp(t)
    return statuses, maskt


def _build_program(statuses, nmask):
    nc = bacc.Bacc(None, target_bir_lowering=False)

    xt_d = nc.dram_tensor("xt", [D, T], BF16, kind="ExternalInput")
    wq_d = nc.dram_tensor("wqt", [D, HPC * HD], BF16, kind="ExternalInput")
    wk_d = nc.dram_tensor("wkt", [D, HPC * HD], BF16, kind="ExternalInput")
    wv_d = nc.dram_tensor("wvt", [D, HPC * HD], BF16, kind="ExternalInput")
    wo_d = nc.dram_tensor("wot", [HPC * HD, D], BF16, kind="ExternalInput")
    cs_d = nc.dram_tensor("cs", [HD, 2, T], F32, kind="ExternalInput")
    mk_d = nc.dram_tensor("maskt", [HD, nmask, PAIR], BF16, kind="ExternalInput")
    ones_d = nc.dram_tensor("ones_col", [HD, 1], BF16, kind="ExternalInput")
    out_d = nc.dram_tensor("out", [T, D], BF16, kind="ExternalOutput")

    xt_ap = xt_d.ap().rearrange("(k p) t -> p k t", p=HD)
    wq_ap = wq_d.ap().rearrange("(k p) e -> p k e", p=HD)
    wk_ap = wk_d.ap().rearrange("(k p) e -> p k e", p=HD)
    wv_ap = wv_d.ap().rearrange("(k p) e -> p k e", p=HD)
    wo_ap = wo_d.ap().rearrange("(h p) e -> p h e", p=HD)
    EXP = mybir.ActivationFunctionType.Exp
    preload_mask = nmask <= MASK_PRELOAD_MAX

    with tile.TileContext(nc) as tc, ExitStack() as top:
        constp = top.enter_context(tc.tile_pool(name="const", bufs=1))
        ones_sb = constp.tile([HD, 1], BF16)
        scr_sb = constp.tile([1, 2], F32)
        nc.scalar.dma_start(ones_sb[:], ones_d[:])
        # dummy exp so the ScalarE activation table for Exp loads during
        # the projection phase instead of stalling the first real exp
        nc.scalar.activation(scr_sb[0:1, 0:1], ones_sb[0:1, 0:1], EXP)

        qkp = top.enter_context(tc.tile_pool(name="qkp", bufs=1))
        # q heads at [:, h, :], k heads at [:, 4+h, :]
        qk_sb = qkp.tile([HD, 2 * HPC, T], BF16)
        vap = top.enter_context(tc.tile_pool(name="vap", bufs=1))
        v_all = vap.tile([HD, NCHUNK, HPC * HD], BF16)

        # ---- fused q/k/v projection pass (+ fused RoPE) ----
        with ExitStack() as ph:
            wp = ph.enter_context(tc.tile_pool(name="wp", side="right", bufs=1))
            wvp = ph.enter_context(tc.tile_pool(name="wvp", side="right", bufs=1))
            xtp = ph.enter_context(tc.tile_pool(name="xtp", side="right", bufs=2))
            csp = ph.enter_context(tc.tile_pool(name="csp", side="right", bufs=2))
            ropep = ph.enter_context(tc.tile_pool(name="ropep", side="right", bufs=2))
            pps = ph.enter_context(tc.tile_pool(name="pps", bufs=5, space="PSUM"))
            vps = ph.enter_context(tc.tile_pool(name="vps", bufs=2, space="PSUM"))
            wqk_sb = wp.tile([HD, 2, NCHUNK, HPC * HD], BF16)
            wv_sb = wvp.tile([HD, NCHUNK, HPC * HD], BF16)
            # weights split per k-chunk (first matmuls start as soon as
            # chunk 0 lands); wq/wk on the scalar hardware queue, wv on
            # the gpsimd software-DGE queue
            for k in range(NCHUNK):
                nc.scalar.dma_start(wqk_sb[:, 0, k, :], wq_ap[:, k, :])
            for k in range(NCHUNK):
                nc.scalar.dma_start(wqk_sb[:, 1, k, :], wk_ap[:, k, :])
            for k in range(NCHUNK):
                nc.gpsimd.dma_start(wv_sb[:, k, :], wv_ap[:, k, :])
            for ns in range(NSLICE):
                tsl = slice(ns * PAIR, (ns + 1) * PAIR)
                xt = xtp.tile([HD, NCHUNK, PAIR], BF16, tag="xt")
                if ns == 0:
                    # quarter the first slice's DMA so matmul 0 starts early
                    for qq in range(4):
                        nc.sync.dma_start(
                            xt[:, qq * 4:(qq + 1) * 4, :],
                            xt_ap[:, qq * 4:(qq + 1) * 4, tsl],
                        )
                else:
                    nc.sync.dma_start(xt[:], xt_ap[:, :, tsl])
                cs_sl = csp.tile([HD, 2, PAIR], F32, tag="cs")
                nc.sync.dma_start(cs_sl[:], cs_d[:, :, tsl])
                for wsel in range(2):
                    for h in range(HPC):
                        ps = pps.tile([HD, PAIR], F32, tag="ps")
                        hs = slice(h * HD, (h + 1) * HD)
                        for k in range(NCHUNK):
                            nc.tensor.matmul(
                                ps[:],
                                wqk_sb[:, wsel, k, hs],
                                xt[:, k, :],
                                start=(k == 0),
                                stop=(k == NCHUNK - 1),
                            )
                        # RoPE: dst = raw*C + swap(raw)*S.  The swap is
                        # materialized by two ScalarE half-copies; DVE does
                        # two multiplies and one add per tile.
                        dst = qk_sb[:, wsel * HPC + h, tsl]
                        sw = ropep.tile([HD, PAIR], F32, tag="sw")
                        nc.scalar.copy(sw[0:64, :], ps[64:128, :])
                        nc.scalar.copy(sw[64:128, :], ps[0:64, :])
                        tb = ropep.tile([HD, PAIR], BF16, tag="tb")
                        nc.vector.tensor_mul(dst, ps[:], cs_sl[:, 0, :])
                        nc.vector.tensor_mul(tb[:], sw[:], cs_sl[:, 1, :])
                        nc.vector.tensor_add(dst, dst, tb[:])
                # v projection for this slice: x^T chunks stationary, so
                # v lands directly in (t x e) layout
                for tc2 in range(2):
                    psv = vps.tile([HD, HPC * HD], F32, tag="vps")
                    for k in range(NCHUNK):
                        nc.tensor.matmul(
                            psv[:],
                            xt[:, k, tc2 * HD:(tc2 + 1) * HD],
                            wv_sb[:, k, :],
                            start=(k == 0),
                            stop=(k == NCHUNK - 1),
                        )
                    nc.scalar.copy(v_all[:, ns * 2 + tc2, :], psv[:])

        # ---- attention ----
        ctxp = top.enter_context(tc.tile_pool(name="ctxp", bufs=1))
        ctx_sb = ctxp.tile([HD, HPC, T], BF16)
        wop = top.enter_context(tc.tile_pool(name="wop", bufs=1))
        wo_sb = wop.tile([HD, HPC, D], BF16)
        with ExitStack() as ph:
            ptp = ph.enter_context(tc.tile_pool(name="ptp", side="right", bufs=3))
            mkp = ph.enter_context(tc.tile_pool(name="mkp", side="right", bufs=4))
            lrp = ph.enter_context(tc.tile_pool(name="lrp", side="right", bufs=2))
            rbp = ph.enter_context(tc.tile_pool(name="rbp", side="right", bufs=2))
            sps = ph.enter_context(tc.tile_pool(name="sps", bufs=4, space="PSUM"))
            ops = ph.enter_context(tc.tile_pool(name="ops", bufs=2, space="PSUM"))
            lps = ph.enter_context(tc.tile_pool(name="lps", bufs=2, space="PSUM"))

            mk_sb = None
            if preload_mask:
                mkpre = ph.enter_context(
                    tc.tile_pool(name="mkpre", side="right", bufs=1)
                )
                mk_sb = mkpre.tile([HD, nmask, PAIR], BF16)
                nc.scalar.dma_start(mk_sb[:], mk_d[:])
            for h in range(HPC):  # prefetch wo
                nc.scalar.dma_start(wo_sb[:, h, :], wo_ap[:, h, :])

            def mask_tile(mi):
                if preload_mask:
                    return mk_sb[:, mi, :]
                mt = mkp.tile([HD, PAIR], BF16, tag="mk")
                nc.scalar.dma_start(mt[:], mk_d[:, mi, :])
                return mt[:]

            def finalize(st):
                # off the tensor engine: DVE fast-recip -> GpSimd partition
                # broadcast -> DVE multiply into ctx
                lr = lrp.tile([1, PAIR], F32, tag="lr")
                nc.vector.reciprocal_approx_fast(lr[:], st["l"])
                rb_sb = rbp.tile([HD, PAIR], F32, tag="rb")
                nc.gpsimd.partition_broadcast(rb_sb[:], lr[:])
                nc.vector.tensor_mul(
                    ctx_sb[:, st["h"], st["qsl"]], st["o"], rb_sb[:]
                )

            def emit_ol(dq):
                # deferred p@v and row-sum matmuls for an exp'd batch.
                # All o matmuls stream first, then all l matmuls, so the
                # PE's moving-operand source stays on pt for the whole
                # run (source switches cost ~95ns each).
                pi, batch, st = dq
                h = st["h"]
                for t, (c, mi) in enumerate(batch):
                    nc.tensor.matmul(
                        st["o"],
                        v_all[:, c, h * HD:(h + 1) * HD],
                        st["pt"][:, pi + t, :],
                        start=(st["oi"] == 0),
                        stop=(st["oi"] == st["n"] - 1),
                        skip_group_check=True,
                    )
                    st["oi"] += 1
                for t, (c, mi) in enumerate(batch):
                    nc.tensor.matmul(
                        st["l"],
                        ones_sb[:],
                        st["pt"][:, pi + t, :],
                        start=(st["li"] == 0),
                        stop=(st["li"] == st["n"] - 1),
                        skip_group_check=True,
                    )
                    st["li"] += 1
                return st["li"] == st["n"]

            pend = deque()   # batches whose ol matmuls are deferred
            fin_q = []       # sts whose ol is fully emitted, finalize pending

            def pop_one():
                dq = pend.popleft()
                if emit_ol(dq):
                    fin_q.append(dq[2])

            def pend_chunks():
                return sum(len(dq[1]) for dq in pend)

            BATCH = 8  # chunks per pipeline batch (4 PSUM score banks)
            for j in range(NPAIR):
                qsl = slice(j * PAIR, (j + 1) * PAIR)
                chunks = statuses[j]  # already in reversed chunk order
                n = len(chunks)
                batches = [chunks[ii:ii + BATCH] for ii in range(0, n, BATCH)]
                for h in range(HPC):
                    o_ps = ops.tile([HD, PAIR], F32, tag="o")
                    l_ps = lps.tile([1, PAIR], F32, tag="l")
                    pt = ptp.tile([HD, NCHUNK, PAIR], BF16, tag="pt")
                    st = {"o": o_ps[:], "l": l_ps[:],
                          "pt": pt, "h": h, "qsl": qsl, "n": n,
                          "oi": 0, "li": 0}
                    pi = 0
                    for batch in batches:
                        w = len(batch)
                        # scores: one long qk_sb-sourced run, split into
                        # per-pair PSUM tiles so exp can chase the matmuls
                        stiles = []
                        for ii in range(0, w, 2):
                            pr = batch[ii:ii + 2]
                            s_ps = sps.tile([HD, 2, PAIR], F32, tag="s")
                            stiles.append((ii, pr, s_ps))
                            for t, (c, mi) in enumerate(pr):
                                nc.tensor.matmul(
                                    s_ps[:, t, :],
                                    qk_sb[:, HPC + h, c * HD:(c + 1) * HD],
                                    qk_sb[:, h, qsl],
                                    start=True,
                                    stop=True,
                                )
                        for ii, pr, s_ps in stiles:
                            wp2 = len(pr)
                            nc.scalar.activation(
                                pt[:, pi + ii:pi + ii + wp2, :],
                                s_ps[:, 0:wp2, :], EXP,
                            )
                            # multiplicative exp-mask applied to pt
                            # (exp(s+m) == exp(s)*exp(m)), off the exp chain
                            t = 0
                            while t < wp2:
                                c, mi = pr[t]
                                if mi < 0:
                                    t += 1
                                    continue
                                r = t + 1
                                while (preload_mask and r < wp2
                                       and pr[r][1] >= 0
                                       and pr[r][1] == pr[r - 1][1] + 1):
                                    r += 1
                                if preload_mask:
                                    sl = slice(pi + ii + t, pi + ii + r)
                                    nc.vector.tensor_mul(
                                        pt[:, sl, :], pt[:, sl, :],
                                        mk_sb[:, mi:mi + (r - t), :],
                                    )
                                else:
                                    sl = slice(pi + ii + t, pi + ii + t + 1)
                                    nc.vector.tensor_mul(
                                        pt[:, sl, :], pt[:, sl, :],
                                        mask_tile(mi),
                                    )
                                    r = t + 1
                                t = r
                        while fin_q:
                            finalize(fin_q.pop(0))
                        # keep ~one full batch of exp'd chunks in flight:
                        # short (small-j) batches accumulate so the PE
                        # always has deferred ol work to hide exp latency
                        while pend_chunks() >= BATCH or len(pend) >= 4:
                            pop_one()
                        pend.append((pi, batch, st))
                        pi += w
            while pend:
                pop_one()
            while fin_q:
                finalize(fin_q.pop(0))
        # ---- output projection (forward order: with ascending-j
        # attention, low t-chunks finished longest ago) ----
        with ExitStack() as ph:
            evp = ph.enter_context(tc.tile_pool(name="evp", side="right", bufs=6))
            wops = ph.enter_context(tc.tile_pool(name="wops", bufs=6, space="PSUM"))
            oqs = [nc.sync, nc.scalar, nc.gpsimd]
            for oi, tck in enumerate(range(NCHUNK)):
                tsl = slice(tck * HD, (tck + 1) * HD)
                for es in range(4):
                    esl = slice(es * 512, (es + 1) * 512)
                    ps = wops.tile([HD, 512], F32, tag="wo")
                    for h in range(HPC):
                        nc.tensor.matmul(
                            ps[:],
                            ctx_sb[:, h, tsl],
                            wo_sb[:, h, esl],
                            start=(h == 0),
                            stop=(h == HPC - 1),
                        )
                    ev = evp.tile([HD, 512], BF16, tag="ev")
                    nc.scalar.copy(ev[:], ps[:])
                    oqs[(oi * 4 + es) % 3].dma_start(out_d[tsl, esl], ev[:])
    nc.compile()
    return nc


_PERM = np.concatenate(
    [np.concatenate([np.arange(0, HD, 2), np.arange(1, HD, 2)]) + h * HD
     for h in range(HPC)]
)


def prepare(x, freqs, mask, wq, wk, wv, wo):
    """Host-side sharding/prep. Returns (nc, in_maps)."""
    x = np.asarray(x, np.float32)
    freqs = np.asarray(freqs, np.float32)
    mask = np.asarray(mask, np.float32)
    wq, wk, wv, wo = (np.asarray(w, np.float32) for w in (wq, wk, wv, wo))

    statuses, maskt = _mask_structure(mask)
    nc = _build_program(statuses, maskt.shape[1])

    scale = np.float32(1.0 / np.sqrt(HD))
    cos = np.ascontiguousarray(freqs[:, :, 0].T)  # (64, T)
    sin = np.ascontiguousarray(freqs[:, :, 1].T)
    cs = np.empty((HD, 2, T), np.float32)
    cs[0:64, 0, :] = cos
    cs[64:128, 0, :] = cos
    cs[0:64, 1, :] = -sin
    cs[64:128, 1, :] = sin

    ones_col = np.ones((HD, 1), bfloat16)
    maskt16 = maskt.astype(bfloat16)
    xt = [np.ascontiguousarray(x[b].T).astype(bfloat16) for b in range(B)]

    in_maps = []
    for core in range(8):
        b, g = core // 4, core % 4
        cols = slice(g * HPC * HD, (g + 1) * HPC * HD)
        in_maps.append({
            "xt": xt[b],
            "wqt": np.ascontiguousarray(
                (wq.T[:, cols] * scale)[:, _PERM]).astype(bfloat16),
            "wkt": np.ascontiguousarray(wk.T[:, cols][:, _PERM]).astype(bfloat16),
            "wvt": np.ascontiguousarray(wv.T[:, cols]).astype(bfloat16),
            "wot": np.ascontiguousarray(wo.T[cols, :]).astype(bfloat16),
            "cs": cs,
            "maskt": maskt16,
            "ones_col": ones_col,
        })
    return nc, in_maps


def run(x, freqs, mask, wq, wk, wv, wo, **spmd_kwargs):
    nc, in_maps = prepare(x, freqs, mask, wq, wk, wv, wo)
    res = run_bass_kernel_spmd(nc, in_maps, list(range(8)), **spmd_kwargs)
    parts = [res.results[c]["out"].astype(np.float32) for c in range(8)]
    out = np.stack([
        parts[b * 4] + parts[b * 4 + 1] + parts[b * 4 + 2] + parts[b * 4 + 3]
        for b in range(B)
    ]).astype(np.float32)
    return out, res


def kernel(x, freqs, mask, wq, wk, wv, wo):
    out, _ = run(x, freqs, mask, wq, wk, wv, wo)
    return out


# revision 21
# speedup vs baseline: 1.0303x; 1.0267x over previous
"""Trainium2 Bass kernel for causal self-attention with RoPE.

Shapes: x (2, 2048, 2048), 16 heads x 128 head_dim.
Sharding: 8 cores = 2 batch x 4 head-groups (4 heads per core).
Each core computes q/k/v projections for its heads, RoPE, causal-masked
softmax attention, and a partial output projection (its head columns of
wo); the host sums the 4 partials per batch element.

Layout strategy (per core):
  - all matmul operands are bf16 (fp32 PSUM accumulation); the loose
    2e-2 tolerance leaves ample margin and bf16 halves DMA traffic and
    eases PE power throttling relative to fp32r.
  - q,k built in transposed layout (head_dim on partitions, t free) so
    RoPE and the score matmuls need no on-device transposes.  The host
    permutes wq/wk columns so RoPE's even/odd pairs become the two
    partition halves, and pre-scales wq by 1/sqrt(head_dim).
  - q/k/v projections run in ONE pass over x^T (v's matmuls use the
    x^T chunks as the stationary operand), so x^T streams from HBM
    once instead of twice.
  - scores computed as s^T (keys x q) per 256-query group; softmax skips
    the max-subtraction (scores are O(1) by construction); row sums via
    an ones-vector matmul; normalization folded into the PSUM eviction.
  - attention runs a 2-deep software pipeline over key-chunk PAIRS so
    the ScalarE exp latency is fully hidden behind the p@v matmuls of
    older pairs; each (query-group, head)'s o and l accumulate in one
    shared PSUM bank to stay within the 8-bank budget.
  - fully-masked key blocks are skipped (host inspects the mask);
    deduplicated exp-mask tiles are multiplied into p only where a
    block is partially masked (exp(s+m) == exp(s)*exp(m)).
  - DMA is spread over four hardware queues (per-queue throughput is
    ~100 GB/s): wq on scalar, wk on vector, wv on gpsimd, x^T/tables on
    sync; the output round-robins over sync/vector/gpsimd.
"""

import sys
from collections import deque
from contextlib import ExitStack

if "/opt/trn_rl_repo" not in sys.path:
    sys.path.insert(0, "/opt/trn_rl_repo")

import numpy as np
from ml_dtypes import bfloat16

import concourse.bacc as bacc
import concourse.mybir as mybir
import concourse.tile as tile
from concourse.bass_utils import run_bass_kernel_spmd

B, T, D, NH, HD = 2, 2048, 2048, 16, 128
HPC = 4              # heads per core
PAIR = 256           # queries per group
NPAIR = T // PAIR    # 8
NCHUNK = T // HD     # 16 key chunks of 128
NSLICE = T // PAIR   # 8 t-slices for projections
BF16 = mybir.dt.bfloat16
F32 = mybir.dt.float32
MASK_PRELOAD_MAX = 24
DEPTH = 2            # attention pair-pipeline lookahead


def _mask_structure(mask):
    """Classify each (query-group, key-chunk) block of the additive mask.

    Returns (statuses, maskt): statuses[j] is a list of
    (chunk, mask_tile_index_or_minus1), in reversed chunk order (the
    attention loop consumes them in that order, so dedup indices of
    adjacent partial blocks come out consecutive), for blocks that must
    be computed; maskt is the packed (128, nmask, 256) array of
    deduplicated transposed exp-mask tiles for partially-masked blocks.
    """
    statuses = []
    tiles = {}
    tile_list = []
    for j in range(NPAIR):
        q = slice(j * PAIR, (j + 1) * PAIR)
        lst = []
        for c in reversed(range(NCHUNK)):
            k = slice(c * HD, (c + 1) * HD)
            sub = mask[q, k]
            if np.all(sub <= -1e8):
                continue
            if np.all(sub == 0.0):
                lst.append((c, -1))
            else:
                key = sub.tobytes()
                mi = tiles.get(key)
                if mi is None:
                    mi = len(tile_list)
                    tiles[key] = mi
                    tile_list.append(np.ascontiguousarray(sub.T))
                lst.append((c, mi))
        assert lst, f"query group {j} has every key block masked"
        statuses.append(lst)
    nmask = max(1, len(tile_list))
    maskt = np.zeros((HD, nmask, PAIR), np.float32)
    for i, t in enumerate(tile_list):
        assert np.all(t <= 64.0), "additive mask too large for exp-mask trick"
        maskt[:, i, :] = np.exp(t)
    return statuses, maskt


def _build_program(statuses, nmask):
    nc = bacc.Bacc(None, target_bir_lowering=False)

    xt_d = nc.dram_tensor("xt", [D, T], BF16, kind="ExternalInput")
    wq_d = nc.dram_tensor("wqt", [D, HPC * HD], BF16, kind="ExternalInput")
    wk_d = nc.dram_tensor("wkt", [D, HPC * HD], BF16, kind="ExternalInput")
    wv_d = nc.dram_tensor("wvt", [D, HPC * HD], BF16, kind="ExternalInput")
    wo_d = nc.dram_tensor("wot", [HPC * HD, D], BF16, kind="ExternalInput")
    cs_d = nc.dram_tensor("cs", [HD, 2, T], F32, kind="ExternalInput")
    mk_d = nc.dram_tensor("maskt", [HD, nmask, PAIR], BF16, kind="ExternalInput")
    ones_d = nc.dram_tensor("ones_col", [HD, 1], BF16, kind="ExternalInput")
    out_d = nc.dram_tensor("out", [T, D], BF16, kind="ExternalOutput")

    xt_ap = xt_d.ap().rearrange("(k p) t -> p k t", p=HD)
    wq_ap = wq_d.ap().rearrange("(k p) e -> p k e", p=HD)
    wk_ap = wk_d.ap().rearrange("(k p) e -> p k e", p=HD)
    wv_ap = wv_d.ap().rearrange("(k p) e -> p k e", p=HD)
    wo_ap = wo_d.ap().rearrange("(h p) e -> p h e", p=HD)
    EXP = mybir.ActivationFunctionType.Exp
    preload_mask = nmask <= MASK_PRELOAD_MAX

    with tile.TileContext(nc) as tc, ExitStack() as top:
        constp = top.enter_context(tc.tile_pool(name="const", bufs=1))
        ones_sb = constp.tile([HD, 1], BF16)
        scr_sb = constp.tile([1, 2], F32)
        scr2_sb = constp.tile([HD, 2], F32)
        nc.scalar.dma_start(ones_sb[:], ones_d[:])
        # dummy exp / partition_broadcast so the ScalarE Exp table load
        # and the GpSimd pool reconfig (~6us) happen during the
        # projection phase instead of stalling the first real softmax
        nc.scalar.activation(scr_sb[0:1, 0:1], ones_sb[0:1, 0:1], EXP)
        nc.gpsimd.partition_broadcast(scr2_sb[:], scr_sb[0:1, :])

        qkp = top.enter_context(tc.tile_pool(name="qkp", bufs=1))
        # q heads at [:, h, :], k heads at [:, 4+h, :]
        qk_sb = qkp.tile([HD, 2 * HPC, T], BF16)
        vap = top.enter_context(tc.tile_pool(name="vap", bufs=1))
        v_all = vap.tile([HD, NCHUNK, HPC * HD], BF16)

        # ---- fused q/k/v projection pass (+ fused RoPE) ----
        with ExitStack() as ph:
            wp = ph.enter_context(tc.tile_pool(name="wp", side="right", bufs=1))
            wvp = ph.enter_context(tc.tile_pool(name="wvp", side="right", bufs=1))
            xtp = ph.enter_context(tc.tile_pool(name="xtp", side="right", bufs=2))
            csp = ph.enter_context(tc.tile_pool(name="csp", side="right", bufs=2))
            ropep = ph.enter_context(tc.tile_pool(name="ropep", side="right", bufs=2))
            pps = ph.enter_context(tc.tile_pool(name="pps", bufs=5, space="PSUM"))
            vps = ph.enter_context(tc.tile_pool(name="vps", bufs=2, space="PSUM"))
            wqk_sb = wp.tile([HD, 2, NCHUNK, HPC * HD], BF16)
            wv_sb = wvp.tile([HD, NCHUNK, HPC * HD], BF16)
            # weights split per k-chunk (first matmuls start as soon as
            # chunk 0 lands); wq/wk on the scalar hardware queue, wv on
            # the gpsimd software-DGE queue
            for k in range(NCHUNK):
                nc.scalar.dma_start(wqk_sb[:, 0, k, :], wq_ap[:, k, :])
            for k in range(NCHUNK):
                nc.scalar.dma_start(wqk_sb[:, 1, k, :], wk_ap[:, k, :])
            for k in range(NCHUNK):
                nc.gpsimd.dma_start(wv_sb[:, k, :], wv_ap[:, k, :])
            for ns in range(NSLICE):
                tsl = slice(ns * PAIR, (ns + 1) * PAIR)
                xt = xtp.tile([HD, NCHUNK, PAIR], BF16, tag="xt")
                if ns == 0:
                    # quarter the first slice's DMA so matmul 0 starts early
                    for qq in range(4):
                        nc.sync.dma_start(
                            xt[:, qq * 4:(qq + 1) * 4, :],
                            xt_ap[:, qq * 4:(qq + 1) * 4, tsl],
                        )
                else:
                    nc.sync.dma_start(xt[:], xt_ap[:, :, tsl])
                cs_sl = csp.tile([HD, 2, PAIR], F32, tag="cs")
                nc.sync.dma_start(cs_sl[:], cs_d[:, :, tsl])
                for wsel in range(2):
                    for h in range(HPC):
                        ps = pps.tile([HD, PAIR], F32, tag="ps")
                        hs = slice(h * HD, (h + 1) * HD)
                        for k in range(NCHUNK):
                            nc.tensor.matmul(
                                ps[:],
                                wqk_sb[:, wsel, k, hs],
                                xt[:, k, :],
                                start=(k == 0),
                                stop=(k == NCHUNK - 1),
                            )
                        # RoPE: dst = raw*C + swap(raw)*S.  The swap is
                        # materialized by two ScalarE half-copies; DVE does
                        # two multiplies and one add per tile.
                        dst = qk_sb[:, wsel * HPC + h, tsl]
                        sw = ropep.tile([HD, PAIR], F32, tag="sw")
                        nc.scalar.copy(sw[0:64, :], ps[64:128, :])
                        nc.scalar.copy(sw[64:128, :], ps[0:64, :])
                        tb = ropep.tile([HD, PAIR], BF16, tag="tb")
                        nc.vector.tensor_mul(dst, ps[:], cs_sl[:, 0, :])
                        nc.vector.tensor_mul(tb[:], sw[:], cs_sl[:, 1, :])
                        nc.vector.tensor_add(dst, dst, tb[:])
                # v projection for this slice: x^T chunks stationary, so
                # v lands directly in (t x e) layout
                for tc2 in range(2):
                    psv = vps.tile([HD, HPC * HD], F32, tag="vps")
                    for k in range(NCHUNK):
                        nc.tensor.matmul(
                            psv[:],
                            xt[:, k, tc2 * HD:(tc2 + 1) * HD],
                            wv_sb[:, k, :],
                            start=(k == 0),
                            stop=(k == NCHUNK - 1),
                        )
                    nc.scalar.copy(v_all[:, ns * 2 + tc2, :], psv[:])

        # ---- attention ----
        ctxp = top.enter_context(tc.tile_pool(name="ctxp", bufs=1))
        ctx_sb = ctxp.tile([HD, HPC, T], BF16)
        wop = top.enter_context(tc.tile_pool(name="wop", bufs=1))
        wo_sb = wop.tile([HD, HPC, D], BF16)
        with ExitStack() as ph:
            ptp = ph.enter_context(tc.tile_pool(name="ptp", side="right", bufs=3))
            mkp = ph.enter_context(tc.tile_pool(name="mkp", side="right", bufs=4))
            lrp = ph.enter_context(tc.tile_pool(name="lrp", side="right", bufs=2))
            rbp = ph.enter_context(tc.tile_pool(name="rbp", side="right", bufs=2))
            sps = ph.enter_context(tc.tile_pool(name="sps", bufs=4, space="PSUM"))
            ops = ph.enter_context(tc.tile_pool(name="ops", bufs=2, space="PSUM"))
            lps = ph.enter_context(tc.tile_pool(name="lps", bufs=2, space="PSUM"))

            mk_sb = None
            if preload_mask:
                mkpre = ph.enter_context(
                    tc.tile_pool(name="mkpre", side="right", bufs=1)
                )
                mk_sb = mkpre.tile([HD, nmask, PAIR], BF16)
                nc.scalar.dma_start(mk_sb[:], mk_d[:])
            for h in range(HPC):  # prefetch wo
                nc.scalar.dma_start(wo_sb[:, h, :], wo_ap[:, h, :])

            def mask_tile(mi):
                if preload_mask:
                    return mk_sb[:, mi, :]
                mt = mkp.tile([HD, PAIR], BF16, tag="mk")
                nc.scalar.dma_start(mt[:], mk_d[:, mi, :])
                return mt[:]

            def finalize(st):
                # off the tensor engine: DVE fast-recip -> GpSimd partition
                # broadcast -> DVE multiply into ctx
                lr = lrp.tile([1, PAIR], F32, tag="lr")
                nc.vector.reciprocal_approx_fast(lr[:], st["l"])
                rb_sb = rbp.tile([HD, PAIR], F32, tag="rb")
                nc.gpsimd.partition_broadcast(rb_sb[:], lr[:])
                nc.vector.tensor_mul(
                    ctx_sb[:, st["h"], st["qsl"]], st["o"], rb_sb[:]
                )

            def emit_ol(dq):
                # deferred p@v and row-sum matmuls for an exp'd batch.
                # All o matmuls stream first, then all l matmuls, so the
                # PE's moving-operand source stays on pt for the whole
                # run (source switches cost ~95ns each).
                pi, batch, st = dq
                h = st["h"]
                for t, (c, mi) in enumerate(batch):
                    nc.tensor.matmul(
                        st["o"],
                        v_all[:, c, h * HD:(h + 1) * HD],
                        st["pt"][:, pi + t, :],
                        start=(st["oi"] == 0),
                        stop=(st["oi"] == st["n"] - 1),
                        skip_group_check=True,
                    )
                    st["oi"] += 1
                for t, (c, mi) in enumerate(batch):
                    nc.tensor.matmul(
                        st["l"],
                        ones_sb[:],
                        st["pt"][:, pi + t, :],
                        start=(st["li"] == 0),
                        stop=(st["li"] == st["n"] - 1),
                        skip_group_check=True,
                    )
                    st["li"] += 1
                return st["li"] == st["n"]

            pend = deque()   # batches whose ol matmuls are deferred
            fin_q = []       # sts whose ol is fully emitted, finalize pending

            def pop_one():
                dq = pend.popleft()
                if emit_ol(dq):
                    fin_q.append(dq[2])

            def pend_chunks():
                return sum(len(dq[1]) for dq in pend)

            BATCH = 8  # chunks per pipeline batch (4 PSUM score banks)
            for j in range(NPAIR):
                qsl = slice(j * PAIR, (j + 1) * PAIR)
                chunks = statuses[j]  # already in reversed chunk order
                n = len(chunks)
                batches = [chunks[ii:ii + BATCH] for ii in range(0, n, BATCH)]
                for h in range(HPC):
                    o_ps = ops.tile([HD, PAIR], F32, tag="o")
                    l_ps = lps.tile([1, PAIR], F32, tag="l")
                    pt = ptp.tile([HD, NCHUNK, PAIR], BF16, tag="pt")
                    st = {"o": o_ps[:], "l": l_ps[:],
                          "pt": pt, "h": h, "qsl": qsl, "n": n,
                          "oi": 0, "li": 0}
                    pi = 0
                    for batch in batches:
                        w = len(batch)
                        # scores: one long qk_sb-sourced run, split into
                        # per-pair PSUM tiles so exp can chase the matmuls
                        stiles = []
                        for ii in range(0, w, 2):
                            pr = batch[ii:ii + 2]
                            s_ps = sps.tile([HD, 2, PAIR], F32, tag="s")
                            stiles.append((ii, pr, s_ps))
                            for t, (c, mi) in enumerate(pr):
                                nc.tensor.matmul(
                                    s_ps[:, t, :],
                                    qk_sb[:, HPC + h, c * HD:(c + 1) * HD],
                                    qk_sb[:, h, qsl],
                                    start=True,
                                    stop=True,
                                )
                        for ii, pr, s_ps in stiles:
                            wp2 = len(pr)
                            nc.scalar.activation(
                                pt[:, pi + ii:pi + ii + wp2, :],
                                s_ps[:, 0:wp2, :], EXP,
                            )
                            # multiplicative exp-mask applied to pt
                            # (exp(s+m) == exp(s)*exp(m)), off the exp chain
                            t = 0
                            while t < wp2:
                                c, mi = pr[t]
                                if mi < 0:
                                    t += 1
                                    continue
                                r = t + 1
                                while (preload_mask and r < wp2
                                       and pr[r][1] >= 0
                                       and pr[r][1] == pr[r - 1][1] + 1):
                                    r += 1
                                if preload_mask:
                                    sl = slice(pi + ii + t, pi + ii + r)
                                    nc.vector.tensor_mul(
                                        pt[:, sl, :], pt[:, sl, :],
                                        mk_sb[:, mi:mi + (r - t), :],
                                    )
                                else:
                                    sl = slice(pi + ii + t, pi + ii + t + 1)
                                    nc.vector.tensor_mul(
                                        pt[:, sl, :], pt[:, sl, :],
                                        mask_tile(mi),
                                    )
                                    r = t + 1
                                t = r
                        while fin_q:
                            finalize(fin_q.pop(0))
                        # keep ~one full batch of exp'd chunks in flight:
                        # short (small-j) batches accumulate so the PE
                        # always has deferred ol work to hide exp latency
                        while pend_chunks() >= BATCH or len(pend) >= 4:
                            pop_one()
                        pend.append((pi, batch, st))
                        pi += w
            while pend:
                pop_one()
            while fin_q:
                finalize(fin_q.pop(0))
        # ---- output projection (forward order: with ascending-j
        # attention, low t-chunks finished longest ago) ----
        with ExitStack() as ph:
            evp = ph.enter_context(tc.tile_pool(name="evp", side="right", bufs=6))
            wops = ph.enter_context(tc.tile_pool(name="wops", bufs=6, space="PSUM"))
            oqs = [nc.sync, nc.scalar, nc.gpsimd]
            for oi, tck in enumerate(range(NCHUNK)):
                tsl = slice(tck * HD, (tck + 1) * HD)
                for es in range(4):
                    esl = slice(es * 512, (es + 1) * 512)
                    ps = wops.tile([HD, 512], F32, tag="wo")
                    for h in range(HPC):
                        nc.tensor.matmul(
                            ps[:],
                            ctx_sb[:, h, tsl],
                            wo_sb[:, h, esl],
                            start=(h == 0),
                            stop=(h == HPC - 1),
                        )
                    ev = evp.tile([HD, 512], BF16, tag="ev")
                    nc.scalar.copy(ev[:], ps[:])
                    oqs[(oi * 4 + es) % 3].dma_start(out_d[tsl, esl], ev[:])
    nc.compile()
    return nc


_PERM = np.concatenate(
    [np.concatenate([np.arange(0, HD, 2), np.arange(1, HD, 2)]) + h * HD
     for h in range(HPC)]
)


def prepare(x, freqs, mask, wq, wk, wv, wo):
    """Host-side sharding/prep. Returns (nc, in_maps)."""
    x = np.asarray(x, np.float32)
    freqs = np.asarray(freqs, np.float32)
    mask = np.asarray(mask, np.float32)
    wq, wk, wv, wo = (np.asarray(w, np.float32) for w in (wq, wk, wv, wo))

    statuses, maskt = _mask_structure(mask)
    nc = _build_program(statuses, maskt.shape[1])

    scale = np.float32(1.0 / np.sqrt(HD))
    cos = np.ascontiguousarray(freqs[:, :, 0].T)  # (64, T)
    sin = np.ascontiguousarray(freqs[:, :, 1].T)
    cs = np.empty((HD, 2, T), np.float32)
    cs[0:64, 0, :] = cos
    cs[64:128, 0, :] = cos
    cs[0:64, 1, :] = -sin
    cs[64:128, 1, :] = sin

    ones_col = np.ones((HD, 1), bfloat16)
    maskt16 = maskt.astype(bfloat16)
    xt = [np.ascontiguousarray(x[b].T).astype(bfloat16) for b in range(B)]

    in_maps = []
    for core in range(8):
        b, g = core // 4, core % 4
        cols = slice(g * HPC * HD, (g + 1) * HPC * HD)
        in_maps.append({
            "xt": xt[b],
            "wqt": np.ascontiguousarray(
                (wq.T[:, cols] * scale)[:, _PERM]).astype(bfloat16),
            "wkt": np.ascontiguousarray(wk.T[:, cols][:, _PERM]).astype(bfloat16),
            "wvt": np.ascontiguousarray(wv.T[:, cols]).astype(bfloat16),
            "wot": np.ascontiguousarray(wo.T[cols, :]).astype(bfloat16),
            "cs": cs,
            "maskt": maskt16,
            "ones_col": ones_col,
        })
    return nc, in_maps


def run(x, freqs, mask, wq, wk, wv, wo, **spmd_kwargs):
    nc, in_maps = prepare(x, freqs, mask, wq, wk, wv, wo)
    res = run_bass_kernel_spmd(nc, in_maps, list(range(8)), **spmd_kwargs)
    parts = [res.results[c]["out"].astype(np.float32) for c in range(8)]
    out = np.stack([
        parts[b * 4] + parts[b * 4 + 1] + parts[b * 4 + 2] + parts[b * 4 + 3]
        for b in range(B)
    ]).astype(np.float32)
    return out, res


def kernel(x, freqs, mask, wq, wk, wv, wo):
    out, _ = run(x, freqs, mask, wq, wk, wv, wo)
    return out
